# revision 21
# baseline (speedup 1.0000x reference)
"""Trainium2 Bass kernel for nn_BlipAttention_75007308857568.

Single-head BLIP attention: B=32, N=1024, C=768, fp32.
  qkv = x @ qkv_w + qkv_b ; q,k,v split
  scores = q @ k.T / sqrt(C) ; attn = softmax(scores)
  out = attn @ v
  y = (out.swapaxes(1,2).reshape(B,N,C)) @ proj_w + proj_b

Sharding: data-parallel over batch B across 8 NeuronCores (4 batches/core).

Math restructuring (exact up to softmax-invariant terms):
  q_n.k_m = x_n (Wq Wk^T) x_m^T + x_n.(Wq bk) + x_m.(Wk bq) + bq.bk
  The x_n.(Wq bk) and bq.bk terms are constant along the softmax axis (m)
  and drop out exactly. With M = Wq @ Wk^T and w = Wk @ bq:
    scoresT[m,n] = (A_n . x_m)/sqrt(C) + (x_m . w)/sqrt(C),  A = x @ M
  K is never computed; the x.w term rides the per-partition bias of the
  exp activation (partition = m). M, w, and the bf16/x16-scaled weight
  copies are one-time host-side transforms; x itself is shipped host-
  transposed ([B, C, N], a layout-only change like the batch sharding)
  so the kernel does no PE transposes of x.

Dtypes: bf16 operands everywhere on the PE (f32 PSUM accumulation).
Measured on hardware, fp8-DoubleRow (despite 2x model throughput) loses
to bf16 here: every non-32-bit matmul carries a mandatory Ldweights
whose cost is not overlapped, the DR moving operand streams slower than
modeled, and fp8 needs residual-compensation passes (3x matmuls) to
pass the 2e-2 gate — while bf16 passes outright (rel_err ~2e-3).
The x16 scale on M/Wv (kept from the fp8 variant) is harmless in bf16:
the exp scale folds S/16 and the 16.0-constant denominator matmul
cancels the V scaling exactly.

Per batch (PE cycles): A 36.9k, V 36.9k, scoresT 49.2k, denom 8.2k,
AV 49.2k, proj 36.9k + P-transposes 6.1k. The next batch's x-load/
bf16-quantize and A/V matmuls weave into the current batch's attention;
the last batch interleaves the projection into the AV stream. The
c-major DRAM scratch between AV and proj implements the reference's
swapaxes+reshape permutation for free.
"""

import math
import os

import numpy as np
import ml_dtypes

import concourse.bacc as bacc
import concourse.bass as bass
import concourse.mybir as mybir
import concourse.tile as tile

from concourse.bass_utils import run_bass_kernel_spmd
from concourse.masks import make_identity

B, N, C = 32, 1024, 768
NCORES = 8
BPC = B // NCORES  # batches per core
CB = C // 128      # 6 channel blocks
NB = N // 128      # 8 sequence blocks
NH = 512           # n-half width (PSUM bank limit for f32)
SCALE = 1.0 / math.sqrt(C)
S16 = SCALE / 16.0
EXPB = -2.0        # exp shift (max logit ~6.73 -> exp(4.73) = 113)

BFNP = ml_dtypes.bfloat16

_CACHE = {}


def _build():
    dt = mybir.dt
    MM = dt.float32r
    f32 = dt.float32
    BF = dt.bfloat16
    SUB = mybir.AluOpType.subtract
    ADD = mybir.AluOpType.add
    MUL = mybir.AluOpType.mult

    nc = bacc.Bacc("TRN2", target_bir_lowering=False, debug=False)

    # x pre-transposed on host: [BPC, C, N]
    xs = nc.dram_tensor("xs", [BPC, C, N], MM, kind="ExternalInput")
    # host-precomputed weights (one-time transforms)
    mbf_d = nc.dram_tensor("mbf", [C, C], MM, kind="ExternalInput")
    wvbf_d = nc.dram_tensor("wvbf", [C, C], MM, kind="ExternalInput")
    wbf_d = nc.dram_tensor("wbf", [C, 128], MM, kind="ExternalInput")
    pw_d = nc.dram_tensor("pw", [C, C], BF, kind="ExternalInput")
    qkv_b = nc.dram_tensor("qkv_b", [3 * C], f32, kind="ExternalInput")
    proj_b = nc.dram_tensor("proj_b", [C], f32, kind="ExternalInput")
    y = nc.dram_tensor("y", [BPC, N, C], f32, kind="ExternalOutput")

    with tile.TileContext(nc) as tc:
        with (
            tc.tile_pool(name="consts", bufs=1) as consts,
            tc.tile_pool(name="wts", bufs=1) as wts,
            tc.tile_pool(name="xp", bufs=2) as xp,
            tc.tile_pool(name="ap", bufs=1) as apool,
            tc.tile_pool(name="vp", bufs=1) as vpool,
            tc.tile_pool(name="ep", bufs=1) as epool,
            tc.tile_pool(name="bwp", bufs=2) as bwp,
            tc.tile_pool(name="rowp", bufs=4) as rowp,
            tc.tile_pool(name="otp", bufs=4) as otp,
            tc.tile_pool(name="rbp", bufs=2) as rbp,
            tc.tile_pool(name="ptp", bufs=4) as ptp,
            tc.tile_pool(name="scrp", bufs=2, space="DRAM") as scrp,
            tc.tile_pool(name="psmm", bufs=4, space="PSUM") as psmm,
            tc.tile_pool(name="psv", bufs=2, space="PSUM") as psv,
            tc.tile_pool(name="pst", bufs=2, space="PSUM") as pst,
        ):
            # ---- constants ----
            ident_f = consts.tile([128, 128], f32, tag="ident_f")
            make_identity(nc, ident_f)
            ident_bf = consts.tile([128, 128], BF, tag="ident_bf")
            nc.vector.tensor_copy(ident_bf, ident_f)

            zero = consts.tile([128, 1], f32, tag="zero")
            nc.gpsimd.memset(zero, 0.0)

            ones11_f = consts.tile([1, 1], f32, tag="o11f")
            nc.gpsimd.memset(ones11_f, 1.0)
            ones11 = consts.tile([1, 1], BF, tag="o11")
            nc.vector.tensor_copy(ones11, ones11_f)

            # 16.0 constant cancels the x16 scaling of Wv in the softmax
            # denominator: recip(16*sum e) * (16 V @ e) == (V@e)/sum e
            ones16_f = consts.tile([128, 128], f32, tag="o16f")
            nc.gpsimd.memset(ones16_f, 16.0)
            ones16 = consts.tile([128, 128], BF, tag="o16")
            nc.gpsimd.tensor_copy(ones16, ones16_f)

            vbp = consts.tile([128, CB], f32, tag="vbp")
            pb = consts.tile([128, C], f32, tag="pb")

            # ---- weights (DMA only) ----
            def ld3(name, dram, dtype):
                t = wts.tile([128, CB, C], dtype, tag=name)
                nc.sync.dma_start(t, dram.ap().rearrange("(cb p) o -> p cb o", p=128))
                return t

            wbf = wts.tile([128, CB, 128], MM, tag="wbf")

            def emit_xt_dma(XTf, b, n0, n1):
                """DMA an n-range of the pre-transposed x into SBUF (SP q)."""
                nc.sync.dma_start(
                    XTf[:, :, n0:n1],
                    xs.ap()[b].rearrange("(cb p) n -> p cb n", p=128)[:, :, n0:n1],
                )

            def emit_bw(XT):
                """bias row: bw16[m] = sum_c w16[c] XT[c,m]; stationary is
                zero-padded to [128,128] (result lands on psum partition 0),
                then 8 tiny matmul-transposes build bwb[m-part, mb]."""
                bw_sb = bwp.tile([1, N], BF, tag="bw_sb")
                for nh in range(2):
                    nsl = slice(nh * NH, (nh + 1) * NH)
                    ps = pst.tile([128, NH], f32, tag="tp")
                    for cb in range(CB):
                        nc.tensor.matmul(
                            ps, wbf[:, cb, :], XT[:, cb, nsl],
                            start=(cb == 0), stop=(cb == CB - 1),
                        )
                    nc.vector.tensor_copy(bw_sb[0:1, nsl], ps[0:1, :])
                psT = pst.tile([128, NH], f32, tag="tp")
                for mb in range(NB):
                    nc.tensor.matmul(
                        psT[:, mb : mb + 1],
                        bw_sb[0:1, mb * 128 : (mb + 1) * 128],
                        ones11, start=True, stop=True,
                    )
                bwb = bwp.tile([128, NB], f32, tag="bwb")
                nc.vector.tensor_scalar(
                    bwb, psT[:, 0:NB], S16, EXPB, op0=MUL, op1=ADD
                )
                return bwb

            def emit_a_half(XT, A, ob, nh):
                """One (ob, n-half) tile of A = x @ 16M (bf16)."""
                obsl = slice(ob * 128, (ob + 1) * 128)
                nsl = slice(nh * NH, (nh + 1) * NH)
                ps = psmm.tile([128, NH], f32, tag="mm")
                for cb in range(CB):
                    nc.tensor.matmul(
                        ps, Mbf[:, cb, obsl], XT[:, cb, nsl],
                        start=(cb == 0), stop=(cb == CB - 1),
                    )
                nc.vector.tensor_copy(A[:, ob, nsl], ps)

            def emit_a(XT, A):
                for ob in range(CB):
                    obsl = slice(ob * 128, (ob + 1) * 128)
                    ps0 = psmm.tile([128, NH], f32, tag="mm")
                    ps1 = psmm.tile([128, NH], f32, tag="mm")
                    for cb in range(CB):
                        st = Mbf[:, cb, obsl]
                        nc.tensor.matmul(ps0, st, XT[:, cb, 0:NH],
                                         start=(cb == 0), stop=(cb == CB - 1))
                        nc.tensor.matmul(ps1, st, XT[:, cb, NH:N],
                                         start=(cb == 0), stop=(cb == CB - 1))
                    nc.vector.tensor_copy(A[:, ob, 0:NH], ps0)
                    nc.vector.tensor_copy(A[:, ob, NH:N], ps1)

            def emit_v_mb(XT, V, mb):
                """V[mb] = x[mb-block] @ 16Wv (bf16)."""
                msl = slice(mb * 128, (mb + 1) * 128)
                psA = psv.tile([128, NH], f32, tag="vmm")
                psB = psv.tile([128, NH], f32, tag="vmm")
                for cb in range(CB):
                    st = XT[:, cb, msl]
                    nc.tensor.matmul(psA, st, Wvbf[:, cb, 0:NH],
                                     start=(cb == 0), stop=(cb == CB - 1))
                    nc.tensor.matmul(psB[:, 0:256], st, Wvbf[:, cb, NH:C],
                                     start=(cb == 0), stop=(cb == CB - 1))
                nc.scalar.add(V[:, mb, 0:NH], psA, zero[:, 0:1])
                nc.scalar.add(V[:, mb, NH:C], psB[:, 0:256], zero[:, 0:1])

            def emit_scores_mb(XT, A, eT, bwb, mb):
                """scoresT [mb, both n-halves] + exp on ACT -> bf16 eT."""
                msl = slice(mb * 128, (mb + 1) * 128)
                ps0 = psmm.tile([128, NH], f32, tag="mm")
                ps1 = psmm.tile([128, NH], f32, tag="mm")
                for cb in range(CB):
                    st = XT[:, cb, msl]
                    nc.tensor.matmul(ps0, st, A[:, cb, 0:NH],
                                     start=(cb == 0), stop=(cb == CB - 1))
                    nc.tensor.matmul(ps1, st, A[:, cb, NH:N],
                                     start=(cb == 0), stop=(cb == CB - 1))
                nc.scalar.activation(
                    eT[:, mb, 0:NH], ps0, mybir.ActivationFunctionType.Exp,
                    scale=S16, bias=bwb[:, mb : mb + 1],
                )
                nc.scalar.activation(
                    eT[:, mb, NH:N], ps1, mybir.ActivationFunctionType.Exp,
                    scale=S16, bias=bwb[:, mb : mb + 1],
                )

            def emit_denom(eT, nh):
                nsl = slice(nh * NH, (nh + 1) * NH)
                dps = psmm.tile([128, NH], f32, tag="mm")
                for mb in range(NB):
                    nc.tensor.matmul(
                        dps, ones16, eT[:, mb, nsl],
                        start=(mb == 0), stop=(mb == NB - 1),
                    )
                rb = rbp.tile([128, NH], f32, tag="rb")
                nc.vector.reciprocal(rb, dps)
                return rb

            def emit_av_cb(V, eT, recips, scrv, cb):
                """OT[cb] both n-halves; DVE normalize; +bv split ACT/DVE
                (exact: softmax weights sum to 1); bf16 scratch."""
                csl = slice(cb * 128, (cb + 1) * 128)
                ps0 = psmm.tile([128, NH], f32, tag="mm")
                ps1 = psmm.tile([128, NH], f32, tag="mm")
                for mb in range(NB):
                    st = V[:, mb, csl]
                    nc.tensor.matmul(ps0, st, eT[:, mb, 0:NH],
                                     start=(mb == 0), stop=(mb == NB - 1))
                    nc.tensor.matmul(ps1, st, eT[:, mb, NH:N],
                                     start=(mb == 0), stop=(mb == NB - 1))
                for nh, ps in ((0, ps0), (1, ps1)):
                    nsl = slice(nh * NH, (nh + 1) * NH)
                    otm = otp.tile([128, NH], BF, tag="ot")
                    nc.vector.tensor_tensor(otm, ps, recips[nh], op=MUL)
                    ot = otp.tile([128, NH], BF, tag="ot")
                    if nh == 0:
                        nc.scalar.add(ot, otm, vbp[:, cb : cb + 1])
                    else:
                        nc.vector.tensor_scalar_add(ot, otm, vbp[:, cb : cb + 1])
                    nc.sync.dma_start(scrv[csl, nsl], ot)

            def emit_prow(scr, ib):
                pview = scr.rearrange("(i j) -> i j", j=C)
                prow = rowp.tile([128, C], BF, tag="prow")
                nc.gpsimd.dma_start(prow, pview[ib * 128 : (ib + 1) * 128, :])
                return prow

            def emit_pj_row(prow, b, ib):
                """One row-block of y = P @ proj_w + proj_b (bf16 core)."""
                pt4a = ptp.tile([128, NH], BF, tag="pt")
                pt4b = ptp.tile([128, NH], BF, tag="pt")
                psA = pst.tile([128, NH], BF, tag="tp")
                for k in range(4):
                    nc.tensor.transpose(
                        psA[:, k * 128 : (k + 1) * 128],
                        prow[:, k * 128 : (k + 1) * 128],
                        ident_bf,
                    )
                nc.vector.tensor_copy(pt4a, psA)
                psB = pst.tile([128, NH], BF, tag="tp")
                for k in range(2):
                    nc.tensor.transpose(
                        psB[:, k * 128 : (k + 1) * 128],
                        prow[:, (4 + k) * 128 : (5 + k) * 128],
                        ident_bf,
                    )
                nc.vector.tensor_copy(pt4b[:, 0:256], psB[:, 0:256])
                ps1 = psmm.tile([128, NH], f32, tag="mm")
                ps2 = psmm.tile([128, NH], f32, tag="mm")
                for jb in range(CB):
                    pt = (pt4a if jb < 4 else pt4b)[
                        :, (jb % 4) * 128 : (jb % 4 + 1) * 128
                    ]
                    nc.tensor.matmul(ps1, pt, PW[:, jb, 0:NH],
                                     start=(jb == 0), stop=(jb == CB - 1))
                    nc.tensor.matmul(ps2[:, 0:256], pt, PW[:, jb, NH:C],
                                     start=(jb == 0), stop=(jb == CB - 1))
                yrow = rowp.tile([128, C], f32, tag="yrow")
                nc.vector.tensor_tensor(yrow[:, 0:NH], ps1, pb[:, 0:NH], op=ADD)
                nc.vector.tensor_tensor(yrow[:, NH:C], ps2[:, 0:256],
                                        pb[:, NH:C], op=ADD)
                nc.scalar.dma_start(y.ap()[b, ib * 128 : (ib + 1) * 128, :], yrow)

            # ---------------- emission schedule ----------------
            import contextlib
            _loop_n = int(os.environ.get("BLIP_LOOP", "0"))
            _loop_ctx = tc.For_i(0, _loop_n, 1) if _loop_n else contextlib.nullcontext()
            _loop_ctx.__enter__()

            def new_x():
                XTf = xp.tile([128, CB, N], MM, tag="XTf")
                return XTf

            # prologue: batch-0 XT halves stream on the SP queue and bf16-
            # quantize as they land; V(mb) follows its chunk; A after; the
            # weights interleave on the same queue.
            XTc = new_x()
            A = apool.tile([128, CB, N], MM, tag="A")
            V = vpool.tile([128, NB, C], BF, tag="V")

            emit_xt_dma(XTc, 0, 0, 128)
            nc.sync.dma_start(wbf, wbf_d.ap().rearrange("(cb p) f -> p cb f", p=128))
            Wvbf = ld3("Wvbf", wvbf_d, MM)
            emit_xt_dma(XTc, 0, 128, NH)
            emit_xt_dma(XTc, 0, NH, N)
            Mbf = ld3("Mbf", mbf_d, MM)
            for k in range(NB):
                emit_v_mb(XTc, V, k)
            nc.sync.dma_start(
                vbp, qkv_b.ap()[2 * C : 3 * C].rearrange("(cb p) -> p cb", p=128)
            )
            for ob in range(CB):
                emit_a_half(XTc, A, ob, 0)
            for ob in range(CB):
                emit_a_half(XTc, A, ob, 1)
            PW = ld3("PW", pw_d, BF)
            nc.sync.dma_start(pb, proj_b.ap()[None, :].to_broadcast([128, C]))
            bwb_c = emit_bw(XTc)

            for b in range(BPC):
                last = b + 1 >= BPC
                if not last:
                    XTn = new_x()
                    emit_xt_dma(XTn, b + 1, 0, NH)
                    emit_xt_dma(XTn, b + 1, NH, N)

                eT = epool.tile([128, NB, N], BF, tag="eT")
                for mb in range(NB):
                    emit_scores_mb(XTc, A, eT, bwb_c, mb)

                recips = [emit_denom(eT, nh) for nh in range(2)]

                # next batch's bias row + A while the normalizers settle
                if not last:
                    bwb_n = emit_bw(XTn)
                    emit_a(XTn, A)

                scr = scrp.tile([C * N], BF, tag="scr")
                scrv = scr.rearrange("(c n) -> c n", n=N)

                if not last:
                    for cb in range(CB):
                        emit_av_cb(V, eT, recips, scrv, cb)
                    prows = [None] * NB
                    prows[0] = emit_prow(scr, 0)
                    prows[1] = emit_prow(scr, 1)
                    for ib in range(NB):
                        emit_v_mb(XTn, V, ib)
                        emit_pj_row(prows[ib], b, ib)
                        if ib + 2 < NB:
                            prows[ib + 2] = emit_prow(scr, ib + 2)
                    XTc, bwb_c = XTn, bwb_n
                else:
                    # epilogue: weave the projection into the AV stream.
                    ready = {0: [0], 1: [1], 2: [2, 3], 3: [4], 4: [5], 5: [6, 7]}
                    prows = {}
                    for cb in range(CB):
                        emit_av_cb(V, eT, recips, scrv, cb)
                        for ib in ready[cb]:
                            prows[ib] = emit_prow(scr, ib)
                        if cb >= 3:
                            for ib in ready[cb - 3]:
                                emit_pj_row(prows[ib], b, ib)
                    for cb in range(CB - 3, CB):
                        for ib in ready[cb]:
                            emit_pj_row(prows[ib], b, ib)

            _loop_ctx.__exit__(None, None, None)

    nc.compile()
    return nc


def _get_nc():
    if "nc" not in _CACHE:
        _CACHE["nc"] = _build()
    return _CACHE["nc"]


def _prep_weights(qkv_w, qkv_b, proj_w):
    """Host-side one-time weight transforms."""
    Wq, Wk, Wv = qkv_w[:, :C], qkv_w[:, C : 2 * C], qkv_w[:, 2 * C :]
    bq = qkv_b[:C]
    mbf = np.ascontiguousarray((16.0 * (Wq @ Wk.T)).astype(np.float32))
    wvbf = np.ascontiguousarray((16.0 * Wv).astype(np.float32))
    w16 = 16.0 * (Wk @ bq)
    wbf = np.zeros((C, 128), dtype=np.float32)
    wbf[:, 0] = w16.astype(np.float32)
    pw = np.ascontiguousarray(proj_w.astype(BFNP))
    return {"mbf": mbf, "wvbf": wvbf, "wbf": wbf, "pw": pw}


def kernel(x, qkv_w, qkv_b, proj_w, proj_b, _trace=False, _tmpdir=None):
    # host-side layout transform: ship x pre-transposed [B, C, N]
    x = np.ascontiguousarray(np.asarray(x, dtype=np.float32).transpose(0, 2, 1))
    qkv_w = np.ascontiguousarray(np.asarray(qkv_w, dtype=np.float32))
    qkv_b = np.ascontiguousarray(np.asarray(qkv_b, dtype=np.float32))
    proj_w = np.ascontiguousarray(np.asarray(proj_w, dtype=np.float32))
    proj_b = np.ascontiguousarray(np.asarray(proj_b, dtype=np.float32))

    shared = _prep_weights(qkv_w, qkv_b, proj_w)
    shared["qkv_b"] = qkv_b
    shared["proj_b"] = proj_b

    nc = _get_nc()
    in_maps = [
        {"xs": x[c * BPC : (c + 1) * BPC], **shared} for c in range(NCORES)
    ]
    res = run_bass_kernel_spmd(
        nc, in_maps, core_ids=list(range(NCORES)),
        trace=_trace, tmpdir=_tmpdir,
        **({"trace_cores": [0]} if _trace else {}),
    )
    out = np.concatenate([res.results[c]["y"] for c in range(NCORES)], axis=0)
    if _trace:
        return out, res
    return out


# revision 22
# speedup vs baseline: 1.0154x; 1.0154x over previous
"""Trainium2 Bass kernel for nn_BlipAttention_75007308857568.

Single-head BLIP attention: B=32, N=1024, C=768, fp32.
  qkv = x @ qkv_w + qkv_b ; q,k,v split
  scores = q @ k.T / sqrt(C) ; attn = softmax(scores)
  out = attn @ v
  y = (out.swapaxes(1,2).reshape(B,N,C)) @ proj_w + proj_b

Sharding: data-parallel over batch B across 8 NeuronCores (4 batches/core).

Math restructuring (exact up to softmax-invariant terms):
  q_n.k_m = x_n (Wq Wk^T) x_m^T + x_n.(Wq bk) + x_m.(Wk bq) + bq.bk
  The x_n.(Wq bk) and bq.bk terms are constant along the softmax axis (m)
  and drop out exactly. With M = Wq @ Wk^T and w = Wk @ bq:
    scoresT[m,n] = (A_n . x_m)/sqrt(C) + (x_m . w)/sqrt(C),  A = x @ M
  K is never computed; the x.w term rides the per-partition bias of the
  exp activation (partition = m). M, w, and the bf16/x16-scaled weight
  copies are one-time host-side transforms; x itself is shipped host-
  transposed ([B, C, N], a layout-only change like the batch sharding)
  so the kernel does no PE transposes of x.

Dtypes, chosen by hardware measurement (loop-delta timing, not the cost
model): the A/V/scores/bw matmuls run on f32r operands — f32r matmuls
SELF-LOAD their weights (no Ldweights instruction) at the same
1 cyc/row for >=256-wide outputs, and on hardware every non-32-bit
matmul carries a mandatory Ldweights whose cost is not overlapped
(~53ns bf16, ~107ns dual-fp8). fp8-DoubleRow loses outright here: DR
Ldweights + slower-than-modeled moving streams + the 3x residual-
compensation passes it needs to pass the 2e-2 gate. AV/denom/proj stay
bf16 (eT/V/P tiles at half SBUF) — an all-f32r version does not fit
SBUF. rel_err ~2.3e-3 vs the 2e-2 gate. The x16 scale on M/Wv (from
the fp8 variant) is harmless: the exp scale folds S/16 and the
16.0-constant denominator matmul cancels the V scaling exactly.

Per batch (PE cycles): A 36.9k, V 36.9k, scoresT 49.2k, denom 8.2k,
AV 49.2k, proj 36.9k + P-transposes 6.1k. The next batch's x-load/
bf16-quantize and A/V matmuls weave into the current batch's attention;
the last batch interleaves the projection into the AV stream. The
c-major DRAM scratch between AV and proj implements the reference's
swapaxes+reshape permutation for free.
"""

import math
import os

import numpy as np
import ml_dtypes

import concourse.bacc as bacc
import concourse.bass as bass
import concourse.mybir as mybir
import concourse.tile as tile

from concourse.bass_utils import run_bass_kernel_spmd
from concourse.masks import make_identity

B, N, C = 32, 1024, 768
NCORES = 8
BPC = B // NCORES  # batches per core
CB = C // 128      # 6 channel blocks
NB = N // 128      # 8 sequence blocks
NH = 512           # n-half width (PSUM bank limit for f32)
SCALE = 1.0 / math.sqrt(C)
S16 = SCALE / 16.0
EXPB = -2.0        # exp shift (max logit ~6.73 -> exp(4.73) = 113)

BFNP = ml_dtypes.bfloat16

_CACHE = {}


def _build():
    dt = mybir.dt
    MM = dt.float32r
    f32 = dt.float32
    BF = dt.bfloat16
    SUB = mybir.AluOpType.subtract
    ADD = mybir.AluOpType.add
    MUL = mybir.AluOpType.mult

    nc = bacc.Bacc("TRN2", target_bir_lowering=False, debug=False)

    # x pre-transposed on host: [BPC, C, N]
    xs = nc.dram_tensor("xs", [BPC, C, N], MM, kind="ExternalInput")
    # host-precomputed weights (one-time transforms)
    mbf_d = nc.dram_tensor("mbf", [C, C], MM, kind="ExternalInput")
    wvbf_d = nc.dram_tensor("wvbf", [C, C], MM, kind="ExternalInput")
    wbf_d = nc.dram_tensor("wbf", [C, 128], MM, kind="ExternalInput")
    pw_d = nc.dram_tensor("pw", [C, C], BF, kind="ExternalInput")
    qkv_b = nc.dram_tensor("qkv_b", [3 * C], f32, kind="ExternalInput")
    proj_b = nc.dram_tensor("proj_b", [C], f32, kind="ExternalInput")
    y = nc.dram_tensor("y", [BPC, N, C], f32, kind="ExternalOutput")

    with tile.TileContext(nc) as tc:
        with (
            tc.tile_pool(name="consts", bufs=1) as consts,
            tc.tile_pool(name="wts", bufs=1) as wts,
            tc.tile_pool(name="xp", bufs=2) as xp,
            tc.tile_pool(name="ap", bufs=1) as apool,
            tc.tile_pool(name="vp", bufs=1) as vpool,
            tc.tile_pool(name="ep", bufs=1) as epool,
            tc.tile_pool(name="bwp", bufs=2) as bwp,
            tc.tile_pool(name="rowp", bufs=4) as rowp,
            tc.tile_pool(name="otp", bufs=4) as otp,
            tc.tile_pool(name="rbp", bufs=2) as rbp,
            tc.tile_pool(name="ptp", bufs=4) as ptp,
            tc.tile_pool(name="scrp", bufs=2, space="DRAM") as scrp,
            tc.tile_pool(name="psmm", bufs=4, space="PSUM") as psmm,
            tc.tile_pool(name="psv", bufs=2, space="PSUM") as psv,
            tc.tile_pool(name="pst", bufs=2, space="PSUM") as pst,
        ):
            # ---- constants ----
            ident_f = consts.tile([128, 128], f32, tag="ident_f")
            make_identity(nc, ident_f)
            ident_bf = consts.tile([128, 128], BF, tag="ident_bf")
            nc.vector.tensor_copy(ident_bf, ident_f)

            zero = consts.tile([128, 1], f32, tag="zero")
            nc.gpsimd.memset(zero, 0.0)

            ones11_f = consts.tile([1, 1], f32, tag="o11f")
            nc.gpsimd.memset(ones11_f, 1.0)
            ones11 = consts.tile([1, 1], BF, tag="o11")
            nc.vector.tensor_copy(ones11, ones11_f)

            # 16.0 constant cancels the x16 scaling of Wv in the softmax
            # denominator: recip(16*sum e) * (16 V @ e) == (V@e)/sum e
            ones16_f = consts.tile([128, 128], f32, tag="o16f")
            nc.gpsimd.memset(ones16_f, 16.0)
            ones16 = consts.tile([128, 128], BF, tag="o16")
            nc.gpsimd.tensor_copy(ones16, ones16_f)

            vbp = consts.tile([128, CB], f32, tag="vbp")
            pb = consts.tile([128, C], f32, tag="pb")

            # ---- weights (DMA only) ----
            def ld3(name, dram, dtype):
                t = wts.tile([128, CB, C], dtype, tag=name)
                nc.sync.dma_start(t, dram.ap().rearrange("(cb p) o -> p cb o", p=128))
                return t

            wbf = wts.tile([128, CB, 128], MM, tag="wbf")

            def emit_xt_dma(XTf, b, n0, n1):
                """DMA an n-range of the pre-transposed x into SBUF (SP q)."""
                nc.sync.dma_start(
                    XTf[:, :, n0:n1],
                    xs.ap()[b].rearrange("(cb p) n -> p cb n", p=128)[:, :, n0:n1],
                )

            def emit_bw(XT):
                """bias row: bw16[m] = sum_c w16[c] XT[c,m]; stationary is
                zero-padded to [128,128] (result lands on psum partition 0),
                then 8 tiny matmul-transposes build bwb[m-part, mb]."""
                bw_sb = bwp.tile([1, N], BF, tag="bw_sb")
                for nh in range(2):
                    nsl = slice(nh * NH, (nh + 1) * NH)
                    ps = pst.tile([128, NH], f32, tag="tp")
                    for cb in range(CB):
                        nc.tensor.matmul(
                            ps, wbf[:, cb, :], XT[:, cb, nsl],
                            start=(cb == 0), stop=(cb == CB - 1),
                        )
                    nc.vector.tensor_copy(bw_sb[0:1, nsl], ps[0:1, :])
                psT = pst.tile([128, NH], f32, tag="tp")
                for mb in range(NB):
                    nc.tensor.matmul(
                        psT[:, mb : mb + 1],
                        bw_sb[0:1, mb * 128 : (mb + 1) * 128],
                        ones11, start=True, stop=True,
                    )
                bwb = bwp.tile([128, NB], f32, tag="bwb")
                nc.vector.tensor_scalar(
                    bwb, psT[:, 0:NB], S16, EXPB, op0=MUL, op1=ADD
                )
                return bwb

            def emit_a_half(XT, A, ob, nh):
                """One (ob, n-half) tile of A = x @ 16M (bf16)."""
                obsl = slice(ob * 128, (ob + 1) * 128)
                nsl = slice(nh * NH, (nh + 1) * NH)
                ps = psmm.tile([128, NH], f32, tag="mm")
                for cb in range(CB):
                    nc.tensor.matmul(
                        ps, Mbf[:, cb, obsl], XT[:, cb, nsl],
                        start=(cb == 0), stop=(cb == CB - 1),
                    )
                nc.vector.tensor_copy(A[:, ob, nsl], ps)

            def emit_a(XT, A):
                for ob in range(CB):
                    obsl = slice(ob * 128, (ob + 1) * 128)
                    ps0 = psmm.tile([128, NH], f32, tag="mm")
                    ps1 = psmm.tile([128, NH], f32, tag="mm")
                    for cb in range(CB):
                        st = Mbf[:, cb, obsl]
                        nc.tensor.matmul(ps0, st, XT[:, cb, 0:NH],
                                         start=(cb == 0), stop=(cb == CB - 1))
                        nc.tensor.matmul(ps1, st, XT[:, cb, NH:N],
                                         start=(cb == 0), stop=(cb == CB - 1))
                    nc.vector.tensor_copy(A[:, ob, 0:NH], ps0)
                    nc.vector.tensor_copy(A[:, ob, NH:N], ps1)

            def emit_v_mb(XT, V, mb):
                """V[mb] = x[mb-block] @ 16Wv (bf16)."""
                msl = slice(mb * 128, (mb + 1) * 128)
                psA = psv.tile([128, NH], f32, tag="vmm")
                psB = psv.tile([128, NH], f32, tag="vmm")
                for cb in range(CB):
                    st = XT[:, cb, msl]
                    nc.tensor.matmul(psA, st, Wvbf[:, cb, 0:NH],
                                     start=(cb == 0), stop=(cb == CB - 1))
                    nc.tensor.matmul(psB[:, 0:256], st, Wvbf[:, cb, NH:C],
                                     start=(cb == 0), stop=(cb == CB - 1))
                nc.scalar.add(V[:, mb, 0:NH], psA, zero[:, 0:1])
                nc.scalar.add(V[:, mb, NH:C], psB[:, 0:256], zero[:, 0:1])

            def emit_scores_mb(XT, A, eT, bwb, mb):
                """scoresT [mb, both n-halves] + exp on ACT -> bf16 eT."""
                msl = slice(mb * 128, (mb + 1) * 128)
                ps0 = psmm.tile([128, NH], f32, tag="mm")
                ps1 = psmm.tile([128, NH], f32, tag="mm")
                for cb in range(CB):
                    st = XT[:, cb, msl]
                    nc.tensor.matmul(ps0, st, A[:, cb, 0:NH],
                                     start=(cb == 0), stop=(cb == CB - 1))
                    nc.tensor.matmul(ps1, st, A[:, cb, NH:N],
                                     start=(cb == 0), stop=(cb == CB - 1))
                nc.scalar.activation(
                    eT[:, mb, 0:NH], ps0, mybir.ActivationFunctionType.Exp,
                    scale=S16, bias=bwb[:, mb : mb + 1],
                )
                nc.scalar.activation(
                    eT[:, mb, NH:N], ps1, mybir.ActivationFunctionType.Exp,
                    scale=S16, bias=bwb[:, mb : mb + 1],
                )

            def emit_denom(eT, nh):
                nsl = slice(nh * NH, (nh + 1) * NH)
                dps = psmm.tile([128, NH], f32, tag="mm")
                for mb in range(NB):
                    nc.tensor.matmul(
                        dps, ones16, eT[:, mb, nsl],
                        start=(mb == 0), stop=(mb == NB - 1),
                    )
                rb = rbp.tile([128, NH], f32, tag="rb")
                nc.vector.reciprocal(rb, dps)
                return rb

            def emit_av_cb(V, eT, recips, scrv, cb):
                """OT[cb] both n-halves; DVE normalize; +bv split ACT/DVE
                (exact: softmax weights sum to 1); bf16 scratch."""
                csl = slice(cb * 128, (cb + 1) * 128)
                ps0 = psmm.tile([128, NH], f32, tag="mm")
                ps1 = psmm.tile([128, NH], f32, tag="mm")
                for mb in range(NB):
                    st = V[:, mb, csl]
                    nc.tensor.matmul(ps0, st, eT[:, mb, 0:NH],
                                     start=(mb == 0), stop=(mb == NB - 1))
                    nc.tensor.matmul(ps1, st, eT[:, mb, NH:N],
                                     start=(mb == 0), stop=(mb == NB - 1))
                for nh, ps in ((0, ps0), (1, ps1)):
                    nsl = slice(nh * NH, (nh + 1) * NH)
                    otm = otp.tile([128, NH], BF, tag="ot")
                    nc.vector.tensor_tensor(otm, ps, recips[nh], op=MUL)
                    ot = otp.tile([128, NH], BF, tag="ot")
                    if nh == 0:
                        nc.scalar.add(ot, otm, vbp[:, cb : cb + 1])
                    else:
                        nc.vector.tensor_scalar_add(ot, otm, vbp[:, cb : cb + 1])
                    nc.sync.dma_start(scrv[csl, nsl], ot)

            def emit_prow(scr, ib):
                pview = scr.rearrange("(i j) -> i j", j=C)
                prow = rowp.tile([128, C], BF, tag="prow")
                nc.gpsimd.dma_start(prow, pview[ib * 128 : (ib + 1) * 128, :])
                return prow

            def emit_pj_row(prow, b, ib):
                """One row-block of y = P @ proj_w + proj_b (bf16 core)."""
                pt4a = ptp.tile([128, NH], BF, tag="pt")
                pt4b = ptp.tile([128, NH], BF, tag="pt")
                psA = pst.tile([128, NH], BF, tag="tp")
                for k in range(4):
                    nc.tensor.transpose(
                        psA[:, k * 128 : (k + 1) * 128],
                        prow[:, k * 128 : (k + 1) * 128],
                        ident_bf,
                    )
                nc.vector.tensor_copy(pt4a, psA)
                psB = pst.tile([128, NH], BF, tag="tp")
                for k in range(2):
                    nc.tensor.transpose(
                        psB[:, k * 128 : (k + 1) * 128],
                        prow[:, (4 + k) * 128 : (5 + k) * 128],
                        ident_bf,
                    )
                nc.vector.tensor_copy(pt4b[:, 0:256], psB[:, 0:256])
                ps1 = psmm.tile([128, NH], f32, tag="mm")
                ps2 = psmm.tile([128, NH], f32, tag="mm")
                for jb in range(CB):
                    pt = (pt4a if jb < 4 else pt4b)[
                        :, (jb % 4) * 128 : (jb % 4 + 1) * 128
                    ]
                    nc.tensor.matmul(ps1, pt, PW[:, jb, 0:NH],
                                     start=(jb == 0), stop=(jb == CB - 1))
                    nc.tensor.matmul(ps2[:, 0:256], pt, PW[:, jb, NH:C],
                                     start=(jb == 0), stop=(jb == CB - 1))
                yrow = rowp.tile([128, C], f32, tag="yrow")
                nc.vector.tensor_tensor(yrow[:, 0:NH], ps1, pb[:, 0:NH], op=ADD)
                nc.vector.tensor_tensor(yrow[:, NH:C], ps2[:, 0:256],
                                        pb[:, NH:C], op=ADD)
                nc.scalar.dma_start(y.ap()[b, ib * 128 : (ib + 1) * 128, :], yrow)

            # ---------------- emission schedule ----------------
            import contextlib
            _loop_n = int(os.environ.get("BLIP_LOOP", "0"))
            _loop_ctx = tc.For_i(0, _loop_n, 1) if _loop_n else contextlib.nullcontext()
            _loop_ctx.__enter__()

            def new_x():
                XTf = xp.tile([128, CB, N], MM, tag="XTf")
                return XTf

            # prologue: batch-0 XT halves stream on the SP queue and bf16-
            # quantize as they land; V(mb) follows its chunk; A after; the
            # weights interleave on the same queue.
            XTc = new_x()
            A = apool.tile([128, CB, N], MM, tag="A")
            V = vpool.tile([128, NB, C], BF, tag="V")

            emit_xt_dma(XTc, 0, 0, 128)
            nc.sync.dma_start(wbf, wbf_d.ap().rearrange("(cb p) f -> p cb f", p=128))
            Wvbf = ld3("Wvbf", wvbf_d, MM)
            emit_xt_dma(XTc, 0, 128, NH)
            emit_xt_dma(XTc, 0, NH, N)
            Mbf = ld3("Mbf", mbf_d, MM)
            for k in range(NB):
                emit_v_mb(XTc, V, k)
            nc.sync.dma_start(
                vbp, qkv_b.ap()[2 * C : 3 * C].rearrange("(cb p) -> p cb", p=128)
            )
            for ob in range(CB):
                emit_a_half(XTc, A, ob, 0)
            for ob in range(CB):
                emit_a_half(XTc, A, ob, 1)
            PW = ld3("PW", pw_d, BF)
            nc.sync.dma_start(pb, proj_b.ap()[None, :].to_broadcast([128, C]))
            bwb_c = emit_bw(XTc)

            for b in range(BPC):
                last = b + 1 >= BPC
                if not last:
                    XTn = new_x()
                    emit_xt_dma(XTn, b + 1, 0, NH)
                    emit_xt_dma(XTn, b + 1, NH, N)

                eT = epool.tile([128, NB, N], BF, tag="eT")
                for mb in range(NB):
                    emit_scores_mb(XTc, A, eT, bwb_c, mb)

                recips = [emit_denom(eT, nh) for nh in range(2)]

                # next batch's bias row + A while the normalizers settle
                if not last:
                    bwb_n = emit_bw(XTn)
                    emit_a(XTn, A)

                scr = scrp.tile([C * N], BF, tag="scr")
                scrv = scr.rearrange("(c n) -> c n", n=N)

                if not last:
                    for cb in range(CB):
                        emit_av_cb(V, eT, recips, scrv, cb)
                    prows = [None] * NB
                    prows[0] = emit_prow(scr, 0)
                    prows[1] = emit_prow(scr, 1)
                    for ib in range(NB):
                        emit_v_mb(XTn, V, ib)
                        emit_pj_row(prows[ib], b, ib)
                        if ib + 2 < NB:
                            prows[ib + 2] = emit_prow(scr, ib + 2)
                    XTc, bwb_c = XTn, bwb_n
                else:
                    # epilogue: weave the projection into the AV stream.
                    ready = {0: [0], 1: [1], 2: [2, 3], 3: [4], 4: [5], 5: [6, 7]}
                    prows = {}
                    for cb in range(CB):
                        emit_av_cb(V, eT, recips, scrv, cb)
                        for ib in ready[cb]:
                            prows[ib] = emit_prow(scr, ib)
                        if cb >= 3:
                            for ib in ready[cb - 3]:
                                emit_pj_row(prows[ib], b, ib)
                    for cb in range(CB - 3, CB):
                        for ib in ready[cb]:
                            emit_pj_row(prows[ib], b, ib)

            _loop_ctx.__exit__(None, None, None)

    nc.compile()
    return nc


def _get_nc():
    if "nc" not in _CACHE:
        _CACHE["nc"] = _build()
    return _CACHE["nc"]


def _prep_weights(qkv_w, qkv_b, proj_w):
    """Host-side one-time weight transforms."""
    Wq, Wk, Wv = qkv_w[:, :C], qkv_w[:, C : 2 * C], qkv_w[:, 2 * C :]
    bq = qkv_b[:C]
    mbf = np.ascontiguousarray((16.0 * (Wq @ Wk.T)).astype(np.float32))
    wvbf = np.ascontiguousarray((16.0 * Wv).astype(np.float32))
    w16 = 16.0 * (Wk @ bq)
    wbf = np.zeros((C, 128), dtype=np.float32)
    wbf[:, 0] = w16.astype(np.float32)
    pw = np.ascontiguousarray(proj_w.astype(BFNP))
    return {"mbf": mbf, "wvbf": wvbf, "wbf": wbf, "pw": pw}


def kernel(x, qkv_w, qkv_b, proj_w, proj_b, _trace=False, _tmpdir=None):
    # host-side layout transform: ship x pre-transposed [B, C, N]
    x = np.ascontiguousarray(np.asarray(x, dtype=np.float32).transpose(0, 2, 1))
    qkv_w = np.ascontiguousarray(np.asarray(qkv_w, dtype=np.float32))
    qkv_b = np.ascontiguousarray(np.asarray(qkv_b, dtype=np.float32))
    proj_w = np.ascontiguousarray(np.asarray(proj_w, dtype=np.float32))
    proj_b = np.ascontiguousarray(np.asarray(proj_b, dtype=np.float32))

    shared = _prep_weights(qkv_w, qkv_b, proj_w)
    shared["qkv_b"] = qkv_b
    shared["proj_b"] = proj_b

    nc = _get_nc()
    in_maps = [
        {"xs": x[c * BPC : (c + 1) * BPC], **shared} for c in range(NCORES)
    ]
    res = run_bass_kernel_spmd(
        nc, in_maps, core_ids=list(range(NCORES)),
        trace=_trace, tmpdir=_tmpdir,
        **({"trace_cores": [0]} if _trace else {}),
    )
    out = np.concatenate([res.results[c]["y"] for c in range(NCORES)], axis=0)
    if _trace:
        return out, res
    return out


# revision 24
# speedup vs baseline: 1.0165x; 1.0011x over previous
"""Trainium2 Bass kernel for nn_BlipAttention_75007308857568.

Single-head BLIP attention: B=32, N=1024, C=768, fp32.
  qkv = x @ qkv_w + qkv_b ; q,k,v split
  scores = q @ k.T / sqrt(C) ; attn = softmax(scores)
  out = attn @ v
  y = (out.swapaxes(1,2).reshape(B,N,C)) @ proj_w + proj_b

Sharding: data-parallel over batch B across 8 NeuronCores (4 batches/core).

Math restructuring (exact up to softmax-invariant terms):
  q_n.k_m = x_n (Wq Wk^T) x_m^T + x_n.(Wq bk) + x_m.(Wk bq) + bq.bk
  The x_n.(Wq bk) and bq.bk terms are constant along the softmax axis (m)
  and drop out exactly. With M = Wq @ Wk^T and w = Wk @ bq:
    scoresT[m,n] = (A_n . x_m)/sqrt(C) + (x_m . w)/sqrt(C),  A = x @ M
  K is never computed; the x.w term rides the per-partition bias of the
  exp activation (partition = m). M, w, and the bf16/x16-scaled weight
  copies are one-time host-side transforms; x itself is shipped host-
  transposed ([B, C, N], a layout-only change like the batch sharding)
  so the kernel does no PE transposes of x.

Dtypes: bf16 operands everywhere on the PE (f32 PSUM accumulation).
Measured on hardware, fp8-DoubleRow (despite 2x model throughput) loses
to bf16 here: every non-32-bit matmul carries a mandatory Ldweights
whose cost is not overlapped, the DR moving operand streams slower than
modeled, and fp8 needs residual-compensation passes (3x matmuls) to
pass the 2e-2 gate — while bf16 passes outright (rel_err ~2e-3).
The x16 scale on M/Wv (kept from the fp8 variant) is harmless in bf16:
the exp scale folds S/16 and the 16.0-constant denominator matmul
cancels the V scaling exactly.

Per batch (PE cycles): A 36.9k, V 36.9k, scoresT 49.2k, denom 8.2k,
AV 49.2k, proj 36.9k + P-transposes 6.1k. The next batch's x-load/
bf16-quantize and A/V matmuls weave into the current batch's attention;
the last batch interleaves the projection into the AV stream. The
c-major DRAM scratch between AV and proj implements the reference's
swapaxes+reshape permutation for free.
"""

import math
import os

import numpy as np
import ml_dtypes

import concourse.bacc as bacc
import concourse.bass as bass
import concourse.mybir as mybir
import concourse.tile as tile

from concourse.bass_utils import run_bass_kernel_spmd
from concourse.masks import make_identity

B, N, C = 32, 1024, 768
NCORES = 8
BPC = B // NCORES  # batches per core
CB = C // 128      # 6 channel blocks
NB = N // 128      # 8 sequence blocks
NH = 512           # n-half width (PSUM bank limit for f32)
SCALE = 1.0 / math.sqrt(C)
S16 = SCALE / 16.0
EXPB = -2.0        # exp shift (max logit ~6.73 -> exp(4.73) = 113)

BFNP = ml_dtypes.bfloat16

_CACHE = {}


def _build():
    dt = mybir.dt
    MM = dt.float32r
    f32 = dt.float32
    BF = dt.bfloat16
    SUB = mybir.AluOpType.subtract
    ADD = mybir.AluOpType.add
    MUL = mybir.AluOpType.mult

    nc = bacc.Bacc("TRN2", target_bir_lowering=False, debug=False)

    # x pre-transposed on host: [BPC, C, N]
    xs = nc.dram_tensor("xs", [BPC, C, N], MM, kind="ExternalInput")
    # host-precomputed weights (one-time transforms)
    mbf_d = nc.dram_tensor("mbf", [C, C], BF, kind="ExternalInput")
    wvbf_d = nc.dram_tensor("wvbf", [C, C], BF, kind="ExternalInput")
    wbf_d = nc.dram_tensor("wbf", [C, 128], BF, kind="ExternalInput")
    pw_d = nc.dram_tensor("pw", [C, C], BF, kind="ExternalInput")
    qkv_b = nc.dram_tensor("qkv_b", [3 * C], f32, kind="ExternalInput")
    proj_b = nc.dram_tensor("proj_b", [C], f32, kind="ExternalInput")
    y = nc.dram_tensor("y", [BPC, N, C], f32, kind="ExternalOutput")

    with tile.TileContext(nc) as tc:
        with (
            tc.tile_pool(name="consts", bufs=1) as consts,
            tc.tile_pool(name="wts", bufs=1) as wts,
            tc.tile_pool(name="xp", bufs=2) as xp,
            tc.tile_pool(name="ap", bufs=1) as apool,
            tc.tile_pool(name="vp", bufs=1) as vpool,
            tc.tile_pool(name="ep", bufs=1) as epool,
            tc.tile_pool(name="bwp", bufs=2) as bwp,
            tc.tile_pool(name="rowp", bufs=4) as rowp,
            tc.tile_pool(name="otp", bufs=4) as otp,
            tc.tile_pool(name="rbp", bufs=2) as rbp,
            tc.tile_pool(name="ptp", bufs=4) as ptp,
            tc.tile_pool(name="scrp", bufs=2, space="DRAM") as scrp,
            tc.tile_pool(name="psmm", bufs=4, space="PSUM") as psmm,
            tc.tile_pool(name="psv", bufs=2, space="PSUM") as psv,
            tc.tile_pool(name="pst", bufs=2, space="PSUM") as pst,
        ):
            # ---- constants ----
            ident_f = consts.tile([128, 128], f32, tag="ident_f")
            make_identity(nc, ident_f)
            ident_bf = consts.tile([128, 128], BF, tag="ident_bf")
            nc.vector.tensor_copy(ident_bf, ident_f)

            zero = consts.tile([128, 1], f32, tag="zero")
            nc.gpsimd.memset(zero, 0.0)

            ones11_f = consts.tile([1, 1], f32, tag="o11f")
            nc.gpsimd.memset(ones11_f, 1.0)
            ones11 = consts.tile([1, 1], BF, tag="o11")
            nc.vector.tensor_copy(ones11, ones11_f)

            # 16.0 constant cancels the x16 scaling of Wv in the softmax
            # denominator: recip(16*sum e) * (16 V @ e) == (V@e)/sum e
            ones16_f = consts.tile([128, 128], f32, tag="o16f")
            nc.gpsimd.memset(ones16_f, 16.0)
            ones16 = consts.tile([128, 128], BF, tag="o16")
            nc.gpsimd.tensor_copy(ones16, ones16_f)

            vbp = consts.tile([128, CB], f32, tag="vbp")
            pb = consts.tile([128, C], f32, tag="pb")

            # ---- weights (DMA only) ----
            def ld3(name, dram, dtype):
                t = wts.tile([128, CB, C], dtype, tag=name)
                nc.sync.dma_start(t, dram.ap().rearrange("(cb p) o -> p cb o", p=128))
                return t

            wbf = wts.tile([128, CB, 128], BF, tag="wbf")

            def emit_xt_dma(XTf, b, n0, n1):
                """DMA an n-range of the pre-transposed x into SBUF (SP q)."""
                nc.sync.dma_start(
                    XTf[:, :, n0:n1],
                    xs.ap()[b].rearrange("(cb p) n -> p cb n", p=128)[:, :, n0:n1],
                )

            def emit_x_chunk_quant(XTf, k, XT):
                """bf16-quantize one n-chunk on ACT (zero-add)."""
                nsl = slice(k * 128, (k + 1) * 128)
                nc.scalar.add(XT[:, :, nsl], XTf[:, :, nsl], zero[:, 0:1])

            def emit_bw(XT):
                """bias row: bw16[m] = sum_c w16[c] XT[c,m]; stationary is
                zero-padded to [128,128] (result lands on psum partition 0),
                then 8 tiny matmul-transposes build bwb[m-part, mb]."""
                bw_sb = bwp.tile([1, N], BF, tag="bw_sb")
                for nh in range(2):
                    nsl = slice(nh * NH, (nh + 1) * NH)
                    ps = pst.tile([128, NH], f32, tag="tp")
                    for cb in range(CB):
                        nc.tensor.matmul(
                            ps, wbf[:, cb, :], XT[:, cb, nsl],
                            start=(cb == 0), stop=(cb == CB - 1),
                        )
                    nc.vector.tensor_copy(bw_sb[0:1, nsl], ps[0:1, :])
                psT = pst.tile([128, NH], f32, tag="tp")
                for mb in range(NB):
                    nc.tensor.matmul(
                        psT[:, mb : mb + 1],
                        bw_sb[0:1, mb * 128 : (mb + 1) * 128],
                        ones11, start=True, stop=True,
                    )
                bwb = bwp.tile([128, NB], f32, tag="bwb")
                nc.vector.tensor_scalar(
                    bwb, psT[:, 0:NB], S16, EXPB, op0=MUL, op1=ADD
                )
                return bwb

            def emit_a_half(XT, A, ob, nh):
                """One (ob, n-half) tile of A = x @ 16M (bf16)."""
                obsl = slice(ob * 128, (ob + 1) * 128)
                nsl = slice(nh * NH, (nh + 1) * NH)
                ps = psmm.tile([128, NH], f32, tag="mm")
                for cb in range(CB):
                    nc.tensor.matmul(
                        ps, Mbf[:, cb, obsl], XT[:, cb, nsl],
                        start=(cb == 0), stop=(cb == CB - 1),
                    )
                nc.scalar.add(A[:, ob, nsl], ps, zero[:, 0:1])

            def emit_a(XT, A):
                for ob in range(CB):
                    obsl = slice(ob * 128, (ob + 1) * 128)
                    ps0 = psmm.tile([128, NH], f32, tag="mm")
                    ps1 = psmm.tile([128, NH], f32, tag="mm")
                    for cb in range(CB):
                        st = Mbf[:, cb, obsl]
                        nc.tensor.matmul(ps0, st, XT[:, cb, 0:NH],
                                         start=(cb == 0), stop=(cb == CB - 1))
                        nc.tensor.matmul(ps1, st, XT[:, cb, NH:N],
                                         start=(cb == 0), stop=(cb == CB - 1))
                    nc.scalar.add(A[:, ob, 0:NH], ps0, zero[:, 0:1])
                    nc.scalar.add(A[:, ob, NH:N], ps1, zero[:, 0:1])

            def emit_v_mb(XT, V, mb):
                """V[mb] = x[mb-block] @ 16Wv (bf16)."""
                msl = slice(mb * 128, (mb + 1) * 128)
                psA = psv.tile([128, NH], f32, tag="vmm")
                psB = psv.tile([128, NH], f32, tag="vmm")
                for cb in range(CB):
                    st = XT[:, cb, msl]
                    nc.tensor.matmul(psA, st, Wvbf[:, cb, 0:NH],
                                     start=(cb == 0), stop=(cb == CB - 1))
                    nc.tensor.matmul(psB[:, 0:256], st, Wvbf[:, cb, NH:C],
                                     start=(cb == 0), stop=(cb == CB - 1))
                nc.scalar.add(V[:, mb, 0:NH], psA, zero[:, 0:1])
                nc.scalar.add(V[:, mb, NH:C], psB[:, 0:256], zero[:, 0:1])

            def emit_scores_mb(XT, A, eT, bwb, mb):
                """scoresT [mb, both n-halves] + exp on ACT -> bf16 eT."""
                msl = slice(mb * 128, (mb + 1) * 128)
                ps0 = psmm.tile([128, NH], f32, tag="mm")
                ps1 = psmm.tile([128, NH], f32, tag="mm")
                for cb in range(CB):
                    st = XT[:, cb, msl]
                    nc.tensor.matmul(ps0, st, A[:, cb, 0:NH],
                                     start=(cb == 0), stop=(cb == CB - 1))
                    nc.tensor.matmul(ps1, st, A[:, cb, NH:N],
                                     start=(cb == 0), stop=(cb == CB - 1))
                nc.scalar.activation(
                    eT[:, mb, 0:NH], ps0, mybir.ActivationFunctionType.Exp,
                    scale=S16, bias=bwb[:, mb : mb + 1],
                )
                nc.scalar.activation(
                    eT[:, mb, NH:N], ps1, mybir.ActivationFunctionType.Exp,
                    scale=S16, bias=bwb[:, mb : mb + 1],
                )

            def emit_denom(eT, nh):
                nsl = slice(nh * NH, (nh + 1) * NH)
                dps = psmm.tile([128, NH], f32, tag="mm")
                for mb in range(NB):
                    nc.tensor.matmul(
                        dps, ones16, eT[:, mb, nsl],
                        start=(mb == 0), stop=(mb == NB - 1),
                    )
                rb = rbp.tile([128, NH], f32, tag="rb")
                nc.vector.reciprocal(rb, dps)
                return rb

            def emit_av_cb(V, eT, recips, scrv, cb):
                """OT[cb] both n-halves; DVE normalize; +bv split ACT/DVE
                (exact: softmax weights sum to 1); bf16 scratch."""
                csl = slice(cb * 128, (cb + 1) * 128)
                ps0 = psmm.tile([128, NH], f32, tag="mm")
                ps1 = psmm.tile([128, NH], f32, tag="mm")
                for mb in range(NB):
                    st = V[:, mb, csl]
                    nc.tensor.matmul(ps0, st, eT[:, mb, 0:NH],
                                     start=(mb == 0), stop=(mb == NB - 1))
                    nc.tensor.matmul(ps1, st, eT[:, mb, NH:N],
                                     start=(mb == 0), stop=(mb == NB - 1))
                for nh, ps in ((0, ps0), (1, ps1)):
                    nsl = slice(nh * NH, (nh + 1) * NH)
                    otm = otp.tile([128, NH], BF, tag="ot")
                    nc.vector.tensor_tensor(otm, ps, recips[nh], op=MUL)
                    ot = otp.tile([128, NH], BF, tag="ot")
                    if nh == 0:
                        nc.scalar.add(ot, otm, vbp[:, cb : cb + 1])
                    else:
                        nc.vector.tensor_scalar_add(ot, otm, vbp[:, cb : cb + 1])
                    nc.sync.dma_start(scrv[csl, nsl], ot)

            def emit_prow(scr, ib):
                pview = scr.rearrange("(i j) -> i j", j=C)
                prow = rowp.tile([128, C], BF, tag="prow")
                nc.gpsimd.dma_start(prow, pview[ib * 128 : (ib + 1) * 128, :])
                return prow

            def emit_pj_row(prow, b, ib):
                """One row-block of y = P @ proj_w + proj_b (bf16 core)."""
                pt4a = ptp.tile([128, NH], BF, tag="pt")
                pt4b = ptp.tile([128, NH], BF, tag="pt")
                psA = pst.tile([128, NH], BF, tag="tp")
                for k in range(4):
                    nc.tensor.transpose(
                        psA[:, k * 128 : (k + 1) * 128],
                        prow[:, k * 128 : (k + 1) * 128],
                        ident_bf,
                    )
                nc.vector.tensor_copy(pt4a, psA)
                psB = pst.tile([128, NH], BF, tag="tp")
                for k in range(2):
                    nc.tensor.transpose(
                        psB[:, k * 128 : (k + 1) * 128],
                        prow[:, (4 + k) * 128 : (5 + k) * 128],
                        ident_bf,
                    )
                nc.vector.tensor_copy(pt4b[:, 0:256], psB[:, 0:256])
                ps1 = psmm.tile([128, NH], f32, tag="mm")
                ps2 = psmm.tile([128, NH], f32, tag="mm")
                for jb in range(CB):
                    pt = (pt4a if jb < 4 else pt4b)[
                        :, (jb % 4) * 128 : (jb % 4 + 1) * 128
                    ]
                    nc.tensor.matmul(ps1, pt, PW[:, jb, 0:NH],
                                     start=(jb == 0), stop=(jb == CB - 1))
                    nc.tensor.matmul(ps2[:, 0:256], pt, PW[:, jb, NH:C],
                                     start=(jb == 0), stop=(jb == CB - 1))
                yrow = rowp.tile([128, C], f32, tag="yrow")
                nc.vector.tensor_tensor(yrow[:, 0:NH], ps1, pb[:, 0:NH], op=ADD)
                nc.vector.tensor_tensor(yrow[:, NH:C], ps2[:, 0:256],
                                        pb[:, NH:C], op=ADD)
                nc.scalar.dma_start(y.ap()[b, ib * 128 : (ib + 1) * 128, :], yrow)

            # ---------------- emission schedule ----------------
            import contextlib
            _loop_n = int(os.environ.get("BLIP_LOOP", "0"))
            _loop_ctx = tc.For_i(0, _loop_n, 1) if _loop_n else contextlib.nullcontext()
            _loop_ctx.__enter__()

            def new_x():
                XTf = xp.tile([128, CB, N], MM, tag="XTf")
                XTt = xp.tile([128, CB, N], BF, tag="XT")
                return XTf, XTt

            # prologue: batch-0 XT halves stream on the SP queue and bf16-
            # quantize as they land; V(mb) follows its chunk; A after; the
            # weights interleave on the same queue.
            XTc_f, XTc = new_x()
            A = apool.tile([128, CB, N], BF, tag="A")
            V = vpool.tile([128, NB, C], BF, tag="V")

            emit_xt_dma(XTc_f, 0, 0, 128)
            # Wv in halves: V(0)'s first-half matmuls start on half 0
            Wvbf = wts.tile([128, CB, C], BF, tag="Wvbf")
            wv_view = wvbf_d.ap().rearrange("(cb p) o -> p cb o", p=128)
            nc.sync.dma_start(Wvbf[:, :, 0:NH], wv_view[:, :, 0:NH])
            emit_xt_dma(XTc_f, 0, 128, NH)
            nc.sync.dma_start(Wvbf[:, :, NH:C], wv_view[:, :, NH:C])
            nc.sync.dma_start(wbf, wbf_d.ap().rearrange("(cb p) f -> p cb f", p=128))
            emit_xt_dma(XTc_f, 0, NH, N)
            Mbf = ld3("Mbf", mbf_d, BF)
            for k in range(NB):
                emit_x_chunk_quant(XTc_f, k, XTc)
                emit_v_mb(XTc, V, k)
            nc.sync.dma_start(
                vbp, qkv_b.ap()[2 * C : 3 * C].rearrange("(cb p) -> p cb", p=128)
            )
            for ob in range(CB):
                emit_a_half(XTc, A, ob, 0)
            for ob in range(CB):
                emit_a_half(XTc, A, ob, 1)
            PW = ld3("PW", pw_d, BF)
            nc.sync.dma_start(pb, proj_b.ap()[None, :].to_broadcast([128, C]))
            bwb_c = emit_bw(XTc)

            for b in range(BPC):
                last = b + 1 >= BPC
                if not last:
                    XTn_f, XTn = new_x()
                    emit_xt_dma(XTn_f, b + 1, 0, NH)
                    emit_xt_dma(XTn_f, b + 1, NH, N)

                # scores with next batch's x quantize woven in
                eT = epool.tile([128, NB, N], BF, tag="eT")
                for mb in range(NB):
                    emit_scores_mb(XTc, A, eT, bwb_c, mb)
                    if not last and mb >= 1:
                        emit_x_chunk_quant(XTn_f, mb - 1, XTn)
                if not last:
                    emit_x_chunk_quant(XTn_f, NB - 1, XTn)

                recips = [emit_denom(eT, nh) for nh in range(2)]

                # next batch's bias row + A while the normalizers settle
                if not last:
                    bwb_n = emit_bw(XTn)
                    emit_a(XTn, A)

                scr = scrp.tile([C * N], BF, tag="scr")
                scrv = scr.rearrange("(c n) -> c n", n=N)

                if not last:
                    for cb in range(CB):
                        emit_av_cb(V, eT, recips, scrv, cb)
                    prows = [None] * NB
                    prows[0] = emit_prow(scr, 0)
                    prows[1] = emit_prow(scr, 1)
                    for ib in range(NB):
                        emit_v_mb(XTn, V, ib)
                        emit_pj_row(prows[ib], b, ib)
                        if ib + 2 < NB:
                            prows[ib + 2] = emit_prow(scr, ib + 2)
                    XTc, bwb_c = XTn, bwb_n
                else:
                    # epilogue: weave the projection into the AV stream.
                    ready = {0: [0], 1: [1], 2: [2, 3], 3: [4], 4: [5], 5: [6, 7]}
                    prows = {}
                    for cb in range(CB):
                        emit_av_cb(V, eT, recips, scrv, cb)
                        for ib in ready[cb]:
                            prows[ib] = emit_prow(scr, ib)
                        if cb >= 3:
                            for ib in ready[cb - 3]:
                                emit_pj_row(prows[ib], b, ib)
                    for cb in range(CB - 3, CB):
                        for ib in ready[cb]:
                            emit_pj_row(prows[ib], b, ib)

            _loop_ctx.__exit__(None, None, None)

    nc.compile()
    return nc


def _get_nc():
    if "nc" not in _CACHE:
        _CACHE["nc"] = _build()
    return _CACHE["nc"]


def _prep_weights(qkv_w, qkv_b, proj_w):
    """Host-side one-time weight transforms."""
    Wq, Wk, Wv = qkv_w[:, :C], qkv_w[:, C : 2 * C], qkv_w[:, 2 * C :]
    bq = qkv_b[:C]
    mbf = np.ascontiguousarray((16.0 * (Wq @ Wk.T)).astype(BFNP))
    wvbf = np.ascontiguousarray((16.0 * Wv).astype(BFNP))
    w16 = 16.0 * (Wk @ bq)
    wbf = np.zeros((C, 128), dtype=BFNP)
    wbf[:, 0] = w16.astype(BFNP)
    pw = np.ascontiguousarray(proj_w.astype(BFNP))
    return {"mbf": mbf, "wvbf": wvbf, "wbf": wbf, "pw": pw}


def kernel(x, qkv_w, qkv_b, proj_w, proj_b, _trace=False, _tmpdir=None):
    # host-side layout transform: ship x pre-transposed [B, C, N]
    x = np.ascontiguousarray(np.asarray(x, dtype=np.float32).transpose(0, 2, 1))
    qkv_w = np.ascontiguousarray(np.asarray(qkv_w, dtype=np.float32))
    qkv_b = np.ascontiguousarray(np.asarray(qkv_b, dtype=np.float32))
    proj_w = np.ascontiguousarray(np.asarray(proj_w, dtype=np.float32))
    proj_b = np.ascontiguousarray(np.asarray(proj_b, dtype=np.float32))

    shared = _prep_weights(qkv_w, qkv_b, proj_w)
    shared["qkv_b"] = qkv_b
    shared["proj_b"] = proj_b

    nc = _get_nc()
    in_maps = [
        {"xs": x[c * BPC : (c + 1) * BPC], **shared} for c in range(NCORES)
    ]
    res = run_bass_kernel_spmd(
        nc, in_maps, core_ids=list(range(NCORES)),
        trace=_trace, tmpdir=_tmpdir,
        **({"trace_cores": [0]} if _trace else {}),
    )
    out = np.concatenate([res.results[c]["y"] for c in range(NCORES)], axis=0)
    if _trace:
        return out, res
    return out
